# revision 36
# baseline (speedup 1.0000x reference)
"""Trainium2 Bass kernel for nn_DilatedMask: 33x33 binary mask dilation.

Computes, for x of shape (8, 2048, 2048, 1) float32:
    mask = (x == 0)
    y    = sliding-window max of mask over a 33x33 window (SAME padding),
           as uint8.

Strategy (per NeuronCore, pure data parallel over the batch of 8):
  A square max window over a binary mask equals (2D box-sum of mask) > 0,
  and the box sum is separable.  Both 1-D 33-wide box sums run on the
  TensorEngine as banded matmuls with the image tile as the *stationary*
  operand, which transposes each pass's output: pass 1 (H-axis sum) emits
  a transposed intermediate [w, h]; pass 2 (W-axis sum over that) lands
  back in natural [h, w] orientation -- no explicit transposes.

    mask  = (x == 0)                              (DVE, f32 -> fp8 {0,1})
    S1^T  = banded sum over H of mask, out [w,h]  (PE, fp8 matmuls)
    S1b   = sign(S1^T) in {0,1}                   (DVE/ACT, PSUM -> fp8)
    S2    = banded sum over W of S1b, out [h,w]   (PE, fp8 matmuls)
    y     = (S2 > 0.5) as uint8                   (DVE/ACT, PSUM -> SBUF)

The H dimension is processed in progressive output bands so pass-2 work
for early rows interleaves with pass-1 of later rows and with the input
DMA stream; the TensorEngine goes dense almost immediately instead of
waiting for the whole image.

PSUM accumulation: within each 512-col PSUM bank the first matmul piece
uses start=True (bank reset), the last stop=True, everything between
accumulates via the per-element has_written bits.
"""

from contextlib import ExitStack

import numpy as np
import ml_dtypes

RADIUS = 16
SE = 2 * RADIUS + 1  # 33
P = 128
BANDW = P + 2 * RADIUS  # 160: out-columns reachable from one 128-row k-tile
BANK = 512  # PSUM bank width in f32 elements
H = W = 2048
N_CORES = 8

# Progressive H output bands: early bands are narrow so pass-1/pass-2 can
# start after only a couple of input row-strips have arrived.
BANDS_H = [(0, 128), (128, 256), (256, 512), (512, 768), (768, 1024),
           (1024, 1536), (1536, 2048)]


def band_np() -> np.ndarray:
    """Band matrix chunk [128, 160]: band[p, j] = 1 iff j-32 <= p <= j."""
    p = np.arange(P)[:, None]
    j = np.arange(BANDW)[None, :]
    return ((p <= j) & (p >= j - 2 * RADIUS)).astype(np.float32)


def _split_at(lo: int, hi: int, cuts):
    out = []
    for c in cuts:
        if lo < c < hi:
            out.append((lo, c))
            lo = c
    out.append((lo, hi))
    return out


def _pieces_for_pass(n: int, extra_cuts=()):
    """Matmul pieces for one banded-sum pass with n output columns.

    Pieces are split at PSUM bank boundaries (512) and any extra cuts.
    Returns list of (kt, lo, hi, start, stop): per physical 512-bank the
    first piece carries start=True, the last stop=True.
    """
    cuts = sorted(set(range(BANK, n, BANK)) | set(extra_cuts))
    nt = n // P
    raw = []  # (kt, lo, hi)
    for kt in range(nt):
        fresh_lo = 0 if kt == 0 else P * kt + RADIUS
        fresh_hi = min(n, P * kt + P + RADIUS)
        for lo, hi in _split_at(fresh_lo, fresh_hi, cuts):
            raw.append((kt, lo, hi))
        if kt > 0:
            for lo, hi in _split_at(P * kt - RADIUS, P * kt + RADIUS, cuts):
                raw.append((kt, lo, hi))
    return raw


def _flag_pieces(raw, tile_base):
    """Assign PSUM start/stop flags for one destination tile's piece list.

    Bank key is relative to the tile base (the tile is bank-aligned); per
    bank the first piece gets start=True, the last stop=True.
    """
    first_in_bank = {}
    last_in_bank = {}
    for i, (kt, lo, hi) in enumerate(raw):
        b = (lo - tile_base) // BANK
        if b not in first_in_bank:
            first_in_bank[b] = i
        last_in_bank[b] = i
    return [
        (kt, lo, hi,
         i == first_in_bank[(lo - tile_base) // BANK],
         i == last_in_bank[(lo - tile_base) // BANK])
        for i, (kt, lo, hi) in enumerate(raw)
    ]


def _dedupe_ldweights(nc):
    """Remove back-to-back duplicate LDWEIGHTS in the PE stream.

    Tile lowers every matmul to LDWEIGHTS+MATMUL; consecutive matmuls that
    share a stationary (the fresh/accumulate piece pairs here) reload the
    identical weights.  The PE pairs each MATMUL with the most recent
    preceding LDWEIGHTS, so the reload is dead -- and LDWEIGHTS streaming
    is this kernel's PE bottleneck.  Only drops loads with empty sync_info.
    """
    import concourse.mybir as mybir

    for blk in nc.m.functions[0].blocks:
        insts = list(blk.instructions)
        keep = []
        remap = {}
        last_sig = None
        for i in insts:
            if i.engine == mybir.EngineType.PE:
                tn = type(i).__name__
                if tn == "InstLdweights":
                    ap = i.ins[0]
                    sig = (ap.memref, ap.offset, str(ap.ap), str(ap.dtype),
                           bool(i.is_transpose), str(i.perf_mode))
                    si = i.sync_info
                    clean = si is None or (
                        not si.on_wait and not si.on_update
                    )
                    if sig == last_sig and clean:
                        remap[i.name] = last_name
                        continue
                    last_sig = sig
                    last_name = i.name
                elif tn != "InstMatmult":
                    last_sig = None
            keep.append(i)
        if remap:
            for i in keep:
                i.remap_dependency_names(remap)
            blk.instructions = keep


def build_program(h: int = H, w: int = W):
    """Build the per-core Bass program (SPMD, identical on all cores)."""
    import concourse.bass as bass
    import concourse.mybir as mybir
    import concourse.tile as tile
    from concourse import bacc

    f32 = mybir.dt.float32
    fp8 = mybir.dt.float8e4
    u8 = mybir.dt.uint8

    nt_h = h // P
    nt_w = w // P
    bands = BANDS_H if h == H else [(0, h)] if h <= 512 else BANDS_H
    if h != H:
        bands = [(a, min(b, h)) for a, b in BANDS_H if a < h]
    n_half = 2 if w > 1024 else 1
    half_w = w // n_half

    nc = bacc.Bacc("TRN2", target_bir_lowering=False, debug=False)
    x_ap = nc.dram_tensor("x", [h, w], f32, kind="ExternalInput").ap()
    band8_ap = nc.dram_tensor("band8", [P, BANDW], fp8, kind="ExternalInput").ap()
    y_ap = nc.dram_tensor("y", [h, w], u8, kind="ExternalOutput").ap()

    band_edges = sorted({a for a, _ in bands} | {b for _, b in bands})
    pieces_h = _pieces_for_pass(h, extra_cuts=band_edges)
    pieces_w = _pieces_for_pass(w, extra_cuts=[half_w] if n_half > 1 else ())
    ph_by_band = {
        bi: _flag_pieces(
            [p for p in pieces_h if blo <= p[1] < bhi], tile_base=blo
        )
        for bi, (blo, bhi) in enumerate(bands)
    }
    pw_by_half = {
        hf: _flag_pieces(
            [p for p in pieces_w if hf * half_w <= p[1] < (hf + 1) * half_w],
            tile_base=hf * half_w,
        )
        for hf in range(n_half)
    }

    OGRP = 2  # output row-strips per store DMA
    n_ogrp = max(1, nt_h // OGRP)
    ogrp = nt_h // n_ogrp

    with tile.TileContext(nc) as tc, ExitStack() as ctx:
        band_pool = ctx.enter_context(tc.tile_pool(name="band", bufs=1))
        xf_pool = ctx.enter_context(tc.tile_pool(name="xf", bufs=6))
        m_pool = ctx.enter_context(tc.tile_pool(name="m", bufs=nt_h))
        s1_pool = ctx.enter_context(tc.tile_pool(name="s1", bufs=nt_w))
        psA_pool = ctx.enter_context(tc.tile_pool(name="psA", bufs=2, space="PSUM"))
        psB_pool = ctx.enter_context(tc.tile_pool(name="psB", bufs=3, space="PSUM"))
        out_pool = ctx.enter_context(tc.tile_pool(name="out", bufs=4))

        band8_t = band_pool.tile([P, BANDW], fp8, tag="band8")
        nc.gpsimd.dma_start(out=band8_t[:], in_=band8_ap[:, :])

        # Persistent PSUM tiles (same memref -> PE WAW stays program-order).
        psA_tiles = [
            psA_pool.tile([P, BANK], f32, tag="psA", name=f"psA{i}")
            for i in range(2)
        ]
        psB_tiles = [
            psB_pool.tile([P, half_w], f32, tag="psB", name=f"psB{i}")
            for i in range(3)
        ]
        nA = [0]
        nB = [0]

        def next_psA():
            t = psA_tiles[nA[0] % len(psA_tiles)]
            nA[0] += 1
            return t

        def next_psB():
            t = psB_tiles[nB[0] % len(psB_tiles)]
            nB[0] += 1
            return t

        # Input: contiguous row strips, masked to fp8 {0,1}.  Masks split
        # between DVE and the otherwise-idle GpSimd (1-input ~= line rate).
        m_tiles = []
        for kt in range(nt_h):
            xf = xf_pool.tile([P, w], f32)
            nc.sync.dma_start(out=xf[:], in_=x_ap[kt * P : (kt + 1) * P, :])
            m = m_pool.tile([P, w], fp8)
            eng = nc.vector if kt % 2 == 0 else nc.gpsimd
            eng.tensor_scalar(
                m[:], xf[:], 0.0, None, mybir.AluOpType.is_equal
            )
            m_tiles.append(m)

        # s1 strips: S1^T[wt] [w', h] in fp8, written band-by-band.
        s1_tiles = [
            s1_pool.tile([P, h], fp8, tag="s1", name=f"s1_{wt}")
            for wt in range(nt_w)
        ]
        yt_tiles = {}
        done_ht = {}

        # PSUM evacuation engine schedules: pass-1 evacs lean DVE (cheaper
        # for narrow tiles), pass-2 thresholds lean ACT (cheaper at 1024).
        ev_flip = [0]
        th_flip = [0]

        def evac(dst_ap, src_ap):
            ev_flip[0] += 1
            if ev_flip[0] % 5 in (0, 1, 3):
                nc.vector.tensor_scalar(
                    dst_ap, src_ap, 0.5, None, mybir.AluOpType.is_gt
                )
            else:
                nc.scalar.sign(dst_ap, src_ap)

        def thresh(dst_ap, src_ap):
            th_flip[0] += 1
            if th_flip[0] % 4 == 0:
                nc.vector.tensor_scalar(
                    dst_ap, src_ap, 0.5, None, mybir.AluOpType.is_gt
                )
            else:
                nc.scalar.sign(dst_ap, src_ap)

        for bi, (blo, bhi) in enumerate(bands):
            bw = bhi - blo
            # Pass 1 for this band: psum cols are h_out in [blo, bhi)
            for wt in range(nt_w):
                psA = next_psA()
                for kt, lo, hi, st, sp in ph_by_band[bi]:
                    base = P * kt - RADIUS
                    nc.tensor.matmul(
                        psA[:, lo - blo : hi - blo],
                        m_tiles[kt][:, wt * P : (wt + 1) * P],
                        band8_t[:, lo - base : hi - base],
                        start=st,
                        stop=sp,
                    )
                evac(s1_tiles[wt][:, blo:bhi], psA[:, :bw])

            # Pass 2 for the band's output row-strips
            for ht in range(blo // P, bhi // P):
                og, a = divmod(ht, ogrp)
                if og not in yt_tiles:
                    yt_tiles[og] = out_pool.tile(
                        [P, ogrp * w], u8, tag="yt", name=f"yt{og}"
                    )
                    done_ht[og] = 0
                yt = yt_tiles[og]
                for hf in range(n_half):
                    psB = next_psB()
                    for wt, lo, hi, st, sp in pw_by_half[hf]:
                        base = P * wt - RADIUS
                        nc.tensor.matmul(
                            psB[:, lo - hf * half_w : hi - hf * half_w],
                            s1_tiles[wt][:, ht * P : (ht + 1) * P],
                            band8_t[:, lo - base : hi - base],
                            start=st,
                            stop=sp,
                        )
                    thresh(
                        yt[:, a * w + hf * half_w : a * w + (hf + 1) * half_w],
                        psB[:, :half_w],
                    )
                done_ht[og] += 1
                if done_ht[og] == ogrp:
                    dst = y_ap[og * ogrp * P : (og + 1) * ogrp * P, :].rearrange(
                        "(a p) w -> p a w", p=P
                    )
                    nc.gpsimd.dma_start(
                        out=dst, in_=yt[:].rearrange("p (a w) -> p a w", a=ogrp)
                    )

    _dedupe_ldweights(nc)
    nc.compile()
    return nc


def kernel(x: np.ndarray) -> np.ndarray:
    """Full-input entry point: x (8, 2048, 2048, 1) f32 -> y same shape uint8."""
    from concourse.bass_utils import run_bass_kernel_spmd

    x = np.asarray(x)
    assert x.shape == (N_CORES, H, W, 1), x.shape
    imgs = np.ascontiguousarray(x[:, :, :, 0], dtype=np.float32)

    nc = build_program(H, W)
    band8 = band_np().astype(ml_dtypes.float8_e4m3)
    in_maps = [{"x": imgs[c], "band8": band8} for c in range(N_CORES)]
    res = run_bass_kernel_spmd(nc, in_maps, list(range(N_CORES)))
    y = np.stack([res.results[c]["y"] for c in range(N_CORES)])
    return y[..., None]


# revision 37
# speedup vs baseline: 2.9165x; 2.9165x over previous
"""Trainium2 Bass kernel for nn_DilatedMask: 33x33 binary mask dilation.

Computes, for x of shape (8, 2048, 2048, 1) float32:
    mask = (x == 0)
    y    = sliding-window max of mask over a 33x33 window (SAME padding),
           as uint8.

Strategy (per NeuronCore, pure data parallel over the batch of 8):
  A square max window over a binary mask equals (2D box-sum of mask) > 0,
  and the box sum is separable.  Both 1-D 33-wide box sums run on the
  TensorEngine as banded matmuls with the image tile as the *stationary*
  operand, which transposes each pass's output: pass 1 (H-axis sum) emits
  a transposed intermediate [w, h]; pass 2 (W-axis sum over that) lands
  back in natural [h, w] orientation -- no explicit transposes.

    mask  = (x == 0)                              (DVE, f32 -> fp8 {0,1})
    S1^T  = banded sum over H of mask, out [w,h]  (PE, fp8 matmuls)
    S1b   = sign(S1^T) in {0,1}                   (DVE/ACT, PSUM -> fp8)
    S2    = banded sum over W of S1b, out [h,w]   (PE, fp8 matmuls)
    y     = (S2 > 0.5) as uint8                   (DVE/ACT, PSUM -> SBUF)

The H dimension is processed in progressive output bands so pass-2 work
for early rows interleaves with pass-1 of later rows and with the input
DMA stream; the TensorEngine goes dense almost immediately instead of
waiting for the whole image.

PSUM accumulation: within each 512-col PSUM bank the first matmul piece
uses start=True (bank reset), the last stop=True, everything between
accumulates via the per-element has_written bits.
"""

from contextlib import ExitStack

import numpy as np
import ml_dtypes

RADIUS = 16
SE = 2 * RADIUS + 1  # 33
P = 128
BANDW = P + 2 * RADIUS  # 160: out-columns reachable from one 128-row k-tile
BANK = 512  # PSUM bank width in f32 elements
H = W = 2048
N_CORES = 8

# Progressive H output bands: early bands are narrow so pass-1/pass-2 can
# start after only a couple of input row-strips have arrived.
BANDS_H = [(0, 128), (128, 256), (256, 512), (512, 768), (768, 1024),
           (1024, 1536), (1536, 2048)]


def band_np() -> np.ndarray:
    """Band matrix chunk [128, 160]: band[p, j] = 1 iff j-32 <= p <= j."""
    p = np.arange(P)[:, None]
    j = np.arange(BANDW)[None, :]
    return ((p <= j) & (p >= j - 2 * RADIUS)).astype(np.float32)


def _split_at(lo: int, hi: int, cuts):
    out = []
    for c in cuts:
        if lo < c < hi:
            out.append((lo, c))
            lo = c
    out.append((lo, hi))
    return out


def _pieces_for_pass(n: int, extra_cuts=()):
    """Matmul pieces for one banded-sum pass with n output columns.

    Pieces are split at PSUM bank boundaries (512) and any extra cuts.
    Returns list of (kt, lo, hi, start, stop): per physical 512-bank the
    first piece carries start=True, the last stop=True.
    """
    cuts = sorted(set(range(BANK, n, BANK)) | set(extra_cuts))
    nt = n // P
    raw = []  # (kt, lo, hi)
    for kt in range(nt):
        fresh_lo = 0 if kt == 0 else P * kt + RADIUS
        fresh_hi = min(n, P * kt + P + RADIUS)
        for lo, hi in _split_at(fresh_lo, fresh_hi, cuts):
            raw.append((kt, lo, hi))
        if kt > 0:
            for lo, hi in _split_at(P * kt - RADIUS, P * kt + RADIUS, cuts):
                raw.append((kt, lo, hi))
    return raw


def _flag_pieces(raw, tile_base):
    """Assign PSUM start/stop flags for one destination tile's piece list.

    Bank key is relative to the tile base (the tile is bank-aligned); per
    bank the first piece gets start=True, the last stop=True.
    """
    first_in_bank = {}
    last_in_bank = {}
    for i, (kt, lo, hi) in enumerate(raw):
        b = (lo - tile_base) // BANK
        if b not in first_in_bank:
            first_in_bank[b] = i
        last_in_bank[b] = i
    return [
        (kt, lo, hi,
         i == first_in_bank[(lo - tile_base) // BANK],
         i == last_in_bank[(lo - tile_base) // BANK])
        for i, (kt, lo, hi) in enumerate(raw)
    ]


def _dedupe_ldweights(nc):
    """Remove back-to-back duplicate LDWEIGHTS in the PE stream.

    Tile lowers every matmul to LDWEIGHTS+MATMUL; consecutive matmuls that
    share a stationary (the fresh/accumulate piece pairs here) reload the
    identical weights.  The PE pairs each MATMUL with the most recent
    preceding LDWEIGHTS, so the reload is dead -- and LDWEIGHTS streaming
    is this kernel's PE bottleneck.  Only drops loads with empty sync_info.
    """
    import concourse.mybir as mybir

    for blk in nc.m.functions[0].blocks:
        insts = list(blk.instructions)
        keep = []
        remap = {}
        last_sig = None
        for i in insts:
            if i.engine == mybir.EngineType.PE:
                tn = type(i).__name__
                if tn == "InstLdweights":
                    ap = i.ins[0]
                    sig = (ap.memref, ap.offset, str(ap.ap), str(ap.dtype),
                           bool(i.is_transpose), str(i.perf_mode))
                    si = i.sync_info
                    clean = si is None or (
                        not si.on_wait and not si.on_update
                    )
                    if sig == last_sig and clean:
                        remap[i.name] = last_name
                        continue
                    last_sig = sig
                    last_name = i.name
                elif tn != "InstMatmult":
                    last_sig = None
            keep.append(i)
        if remap:
            for i in keep:
                i.remap_dependency_names(remap)
            blk.instructions = keep


def build_program(h: int = H, w: int = W):
    """Build the per-core Bass program (SPMD, identical on all cores)."""
    import concourse.bass as bass
    import concourse.mybir as mybir
    import concourse.tile as tile
    from concourse import bacc

    f32 = mybir.dt.float32
    fp8 = mybir.dt.float8e4
    u8 = mybir.dt.uint8

    nt_h = h // P
    nt_w = w // P
    bands = BANDS_H if h == H else [(0, h)] if h <= 512 else BANDS_H
    if h != H:
        bands = [(a, min(b, h)) for a, b in BANDS_H if a < h]
    n_half = 2 if w > 1024 else 1
    half_w = w // n_half

    nc = bacc.Bacc("TRN2", target_bir_lowering=False, debug=False)
    x_ap = nc.dram_tensor("x", [h, w], f32, kind="ExternalInput").ap()
    band8_ap = nc.dram_tensor("band8", [P, BANDW], fp8, kind="ExternalInput").ap()
    y_ap = nc.dram_tensor("y", [h, w], u8, kind="ExternalOutput").ap()

    band_edges = sorted({a for a, _ in bands} | {b for _, b in bands})
    pieces_h = _pieces_for_pass(h, extra_cuts=band_edges)
    pieces_w = _pieces_for_pass(w, extra_cuts=[half_w] if n_half > 1 else ())
    ph_by_band = {
        bi: _flag_pieces(
            [p for p in pieces_h if blo <= p[1] < bhi], tile_base=blo
        )
        for bi, (blo, bhi) in enumerate(bands)
    }
    pw_by_half = {
        hf: _flag_pieces(
            [p for p in pieces_w if hf * half_w <= p[1] < (hf + 1) * half_w],
            tile_base=hf * half_w,
        )
        for hf in range(n_half)
    }

    OGRP = 2  # output row-strips per store DMA
    n_ogrp = max(1, nt_h // OGRP)
    ogrp = nt_h // n_ogrp

    with tile.TileContext(nc) as tc, ExitStack() as ctx:
        band_pool = ctx.enter_context(tc.tile_pool(name="band", bufs=1))
        xf_pool = ctx.enter_context(tc.tile_pool(name="xf", bufs=6))
        m_pool = ctx.enter_context(tc.tile_pool(name="m", bufs=nt_h))
        s1_pool = ctx.enter_context(tc.tile_pool(name="s1", bufs=nt_w))
        psA_pool = ctx.enter_context(tc.tile_pool(name="psA", bufs=2, space="PSUM"))
        psB_pool = ctx.enter_context(tc.tile_pool(name="psB", bufs=3, space="PSUM"))
        out_pool = ctx.enter_context(tc.tile_pool(name="out", bufs=4))

        band8_t = band_pool.tile([P, BANDW], fp8, tag="band8")
        nc.gpsimd.dma_start(out=band8_t[:], in_=band8_ap[:, :])

        # Persistent PSUM tiles (same memref -> PE WAW stays program-order).
        psA_tiles = [
            psA_pool.tile([P, BANK], f32, tag="psA", name=f"psA{i}")
            for i in range(2)
        ]
        psB_tiles = [
            psB_pool.tile([P, half_w], f32, tag="psB", name=f"psB{i}")
            for i in range(3)
        ]
        nA = [0]
        nB = [0]

        def next_psA():
            t = psA_tiles[nA[0] % len(psA_tiles)]
            nA[0] += 1
            return t

        def next_psB():
            t = psB_tiles[nB[0] % len(psB_tiles)]
            nB[0] += 1
            return t

        # Input: contiguous row strips, masked to fp8 {0,1}.  Masks split
        # between DVE and the otherwise-idle GpSimd (1-input ~= line rate).
        m_tiles = []
        for kt in range(nt_h):
            xf = xf_pool.tile([P, w], f32)
            nc.sync.dma_start(out=xf[:], in_=x_ap[kt * P : (kt + 1) * P, :])
            m = m_pool.tile([P, w], fp8)
            nc.vector.tensor_scalar(
                m[:], xf[:], 0.0, None, mybir.AluOpType.is_equal
            )
            m_tiles.append(m)

        # s1 strips: S1^T[wt] [w', h] in fp8, written band-by-band.
        s1_tiles = [
            s1_pool.tile([P, h], fp8, tag="s1", name=f"s1_{wt}")
            for wt in range(nt_w)
        ]
        yt_tiles = {}
        done_ht = {}

        # PSUM evacuation engine schedules: pass-1 evacs lean DVE (cheaper
        # for narrow tiles), pass-2 thresholds lean ACT (cheaper at 1024).
        ev_flip = [0]
        th_flip = [0]

        def evac(dst_ap, src_ap):
            ev_flip[0] += 1
            if ev_flip[0] % 5 in (0, 1, 3):
                nc.vector.tensor_scalar(
                    dst_ap, src_ap, 0.5, None, mybir.AluOpType.is_gt
                )
            else:
                nc.scalar.sign(dst_ap, src_ap)

        def thresh(dst_ap, src_ap):
            th_flip[0] += 1
            if th_flip[0] % 4 == 0:
                nc.vector.tensor_scalar(
                    dst_ap, src_ap, 0.5, None, mybir.AluOpType.is_gt
                )
            else:
                nc.scalar.sign(dst_ap, src_ap)

        for bi, (blo, bhi) in enumerate(bands):
            bw = bhi - blo
            # Pass 1 for this band: psum cols are h_out in [blo, bhi)
            for wt in range(nt_w):
                psA = next_psA()
                for kt, lo, hi, st, sp in ph_by_band[bi]:
                    base = P * kt - RADIUS
                    nc.tensor.matmul(
                        psA[:, lo - blo : hi - blo],
                        m_tiles[kt][:, wt * P : (wt + 1) * P],
                        band8_t[:, lo - base : hi - base],
                        start=st,
                        stop=sp,
                    )
                evac(s1_tiles[wt][:, blo:bhi], psA[:, :bw])

            # Pass 2 for the band's output row-strips
            for ht in range(blo // P, bhi // P):
                og, a = divmod(ht, ogrp)
                if og not in yt_tiles:
                    yt_tiles[og] = out_pool.tile(
                        [P, ogrp * w], u8, tag="yt", name=f"yt{og}"
                    )
                    done_ht[og] = 0
                yt = yt_tiles[og]
                for hf in range(n_half):
                    psB = next_psB()
                    for wt, lo, hi, st, sp in pw_by_half[hf]:
                        base = P * wt - RADIUS
                        nc.tensor.matmul(
                            psB[:, lo - hf * half_w : hi - hf * half_w],
                            s1_tiles[wt][:, ht * P : (ht + 1) * P],
                            band8_t[:, lo - base : hi - base],
                            start=st,
                            stop=sp,
                        )
                    thresh(
                        yt[:, a * w + hf * half_w : a * w + (hf + 1) * half_w],
                        psB[:, :half_w],
                    )
                done_ht[og] += 1
                if done_ht[og] == ogrp:
                    dst = y_ap[og * ogrp * P : (og + 1) * ogrp * P, :].rearrange(
                        "(a p) w -> p a w", p=P
                    )
                    nc.gpsimd.dma_start(
                        out=dst, in_=yt[:].rearrange("p (a w) -> p a w", a=ogrp)
                    )

    _dedupe_ldweights(nc)
    nc.compile()
    return nc


def kernel(x: np.ndarray) -> np.ndarray:
    """Full-input entry point: x (8, 2048, 2048, 1) f32 -> y same shape uint8."""
    from concourse.bass_utils import run_bass_kernel_spmd

    x = np.asarray(x)
    assert x.shape == (N_CORES, H, W, 1), x.shape
    imgs = np.ascontiguousarray(x[:, :, :, 0], dtype=np.float32)

    nc = build_program(H, W)
    band8 = band_np().astype(ml_dtypes.float8_e4m3)
    in_maps = [{"x": imgs[c], "band8": band8} for c in range(N_CORES)]
    res = run_bass_kernel_spmd(nc, in_maps, list(range(N_CORES)))
    y = np.stack([res.results[c]["y"] for c in range(N_CORES)])
    return y[..., None]


# revision 40
# speedup vs baseline: 3.1038x; 1.0642x over previous
"""Trainium2 Bass kernel for nn_DilatedMask: 33x33 binary mask dilation.

Computes, for x of shape (8, 2048, 2048, 1) float32:
    mask = (x == 0)
    y    = sliding-window max of mask over a 33x33 window (SAME padding),
           as uint8.

Strategy (per NeuronCore, pure data parallel over the batch of 8):
  A square max window over a binary mask equals (2D box-sum of mask) > 0,
  and the box sum is separable.  Both 1-D 33-wide box sums run on the
  TensorEngine as banded matmuls with the image tile as the *stationary*
  operand, which transposes each pass's output: pass 1 (H-axis sum) emits
  a transposed intermediate [w, h]; pass 2 (W-axis sum over that) lands
  back in natural [h, w] orientation -- no explicit transposes.

    mask  = (x == 0)                              (DVE, f32 -> fp8 {0,1})
    S1^T  = banded sum over H of mask, out [w,h]  (PE, fp8 matmuls)
    S1b   = sign(S1^T) in {0,1}                   (DVE/ACT, PSUM -> fp8)
    S2    = banded sum over W of S1b, out [h,w]   (PE, fp8 matmuls)
    y     = (S2 > 0.5) as uint8                   (DVE/ACT, PSUM -> SBUF)

The H dimension is processed in progressive output bands so pass-2 work
for early rows interleaves with pass-1 of later rows and with the input
DMA stream; the TensorEngine goes dense almost immediately instead of
waiting for the whole image.

PSUM accumulation: within each 512-col PSUM bank the first matmul piece
uses start=True (bank reset), the last stop=True, everything between
accumulates via the per-element has_written bits.
"""

from contextlib import ExitStack

import numpy as np
import ml_dtypes

RADIUS = 16
SE = 2 * RADIUS + 1  # 33
P = 128
BANDW = P + 2 * RADIUS  # 160: out-columns reachable from one 128-row k-tile
BANK = 512  # PSUM bank width in f32 elements
H = W = 2048
N_CORES = 8

# Progressive H output bands: early bands are narrow so pass-1/pass-2 can
# start after only a couple of input row-strips have arrived.
BANDS_H = [(0, 128), (128, 256), (256, 512), (512, 768), (768, 1024),
           (1024, 1536), (1536, 2048)]


def band_np() -> np.ndarray:
    """Band matrix chunk [128, 160]: band[p, j] = 1 iff j-32 <= p <= j."""
    p = np.arange(P)[:, None]
    j = np.arange(BANDW)[None, :]
    return ((p <= j) & (p >= j - 2 * RADIUS)).astype(np.float32)


def _split_at(lo: int, hi: int, cuts):
    out = []
    for c in cuts:
        if lo < c < hi:
            out.append((lo, c))
            lo = c
    out.append((lo, hi))
    return out


def _pieces_for_pass(n: int, extra_cuts=()):
    """Matmul pieces for one banded-sum pass with n output columns.

    Pieces are split at PSUM bank boundaries (512) and any extra cuts.
    Returns list of (kt, lo, hi, start, stop): per physical 512-bank the
    first piece carries start=True, the last stop=True.
    """
    cuts = sorted(set(range(BANK, n, BANK)) | set(extra_cuts))
    nt = n // P
    raw = []  # (kt, lo, hi)
    for kt in range(nt):
        fresh_lo = 0 if kt == 0 else P * kt + RADIUS
        fresh_hi = min(n, P * kt + P + RADIUS)
        for lo, hi in _split_at(fresh_lo, fresh_hi, cuts):
            raw.append((kt, lo, hi))
        if kt > 0:
            for lo, hi in _split_at(P * kt - RADIUS, P * kt + RADIUS, cuts):
                raw.append((kt, lo, hi))
    return raw


def _flag_pieces(raw, tile_base):
    """Assign PSUM start/stop flags for one destination tile's piece list.

    Bank key is relative to the tile base (the tile is bank-aligned); per
    bank the first piece gets start=True, the last stop=True.
    """
    first_in_bank = {}
    last_in_bank = {}
    for i, (kt, lo, hi) in enumerate(raw):
        b = (lo - tile_base) // BANK
        if b not in first_in_bank:
            first_in_bank[b] = i
        last_in_bank[b] = i
    return [
        (kt, lo, hi,
         i == first_in_bank[(lo - tile_base) // BANK],
         i == last_in_bank[(lo - tile_base) // BANK])
        for i, (kt, lo, hi) in enumerate(raw)
    ]


def _dedupe_ldweights(nc):
    """Remove back-to-back duplicate LDWEIGHTS in the PE stream.

    Tile lowers every matmul to LDWEIGHTS+MATMUL; consecutive matmuls that
    share a stationary (the fresh/accumulate piece pairs here) reload the
    identical weights.  The PE pairs each MATMUL with the most recent
    preceding LDWEIGHTS, so the reload is dead -- and LDWEIGHTS streaming
    is this kernel's PE bottleneck.  Only drops loads with empty sync_info.
    """
    import concourse.mybir as mybir

    for blk in nc.m.functions[0].blocks:
        insts = list(blk.instructions)
        keep = []
        remap = {}
        last_sig = None
        for i in insts:
            if i.engine == mybir.EngineType.PE:
                tn = type(i).__name__
                if tn == "InstLdweights":
                    ap = i.ins[0]
                    sig = (ap.memref, ap.offset, str(ap.ap), str(ap.dtype),
                           bool(i.is_transpose), str(i.perf_mode))
                    si = i.sync_info
                    clean = si is None or (
                        not si.on_wait and not si.on_update
                    )
                    if sig == last_sig and clean:
                        remap[i.name] = last_name
                        continue
                    last_sig = sig
                    last_name = i.name
                elif tn != "InstMatmult":
                    last_sig = None
            keep.append(i)
        if remap:
            for i in keep:
                i.remap_dependency_names(remap)
            blk.instructions = keep


def build_program(h: int = H, w: int = W):
    """Build the per-core Bass program (SPMD, identical on all cores)."""
    import concourse.bass as bass
    import concourse.mybir as mybir
    import concourse.tile as tile
    from concourse import bacc

    f32 = mybir.dt.float32
    fp8 = mybir.dt.float8e4
    u8 = mybir.dt.uint8

    nt_h = h // P
    nt_w = w // P
    bands = BANDS_H if h == H else [(0, h)] if h <= 512 else BANDS_H
    if h != H:
        bands = [(a, min(b, h)) for a, b in BANDS_H if a < h]
    n_half = 2 if w > 1024 else 1
    half_w = w // n_half

    nc = bacc.Bacc("TRN2", target_bir_lowering=False, debug=False)
    x_ap = nc.dram_tensor("x", [h, w], f32, kind="ExternalInput").ap()
    band8_ap = nc.dram_tensor("band8", [P, BANDW], fp8, kind="ExternalInput").ap()
    y_ap = nc.dram_tensor("y", [h, w], u8, kind="ExternalOutput").ap()

    band_edges = sorted({a for a, _ in bands} | {b for _, b in bands})
    pieces_h = _pieces_for_pass(h, extra_cuts=band_edges)
    pieces_w = _pieces_for_pass(w, extra_cuts=[half_w] if n_half > 1 else ())
    ph_by_band = {
        bi: _flag_pieces(
            [p for p in pieces_h if blo <= p[1] < bhi], tile_base=blo
        )
        for bi, (blo, bhi) in enumerate(bands)
    }
    pw_by_half = {
        hf: _flag_pieces(
            [p for p in pieces_w if hf * half_w <= p[1] < (hf + 1) * half_w],
            tile_base=hf * half_w,
        )
        for hf in range(n_half)
    }

    OGRP = 2  # output row-strips per store DMA
    n_ogrp = max(1, nt_h // OGRP)
    ogrp = nt_h // n_ogrp

    with tile.TileContext(nc) as tc, ExitStack() as ctx:
        band_pool = ctx.enter_context(tc.tile_pool(name="band", bufs=1))
        xf_pool = ctx.enter_context(tc.tile_pool(name="xf", bufs=6))
        m_pool = ctx.enter_context(tc.tile_pool(name="m", bufs=nt_h))
        s1_pool = ctx.enter_context(tc.tile_pool(name="s1", bufs=nt_w))
        psA_pool = ctx.enter_context(tc.tile_pool(name="psA", bufs=3, space="PSUM"))
        psB_pool = ctx.enter_context(tc.tile_pool(name="psB", bufs=2, space="PSUM"))
        out_pool = ctx.enter_context(tc.tile_pool(name="out", bufs=4))

        band8_t = band_pool.tile([P, BANDW], fp8, tag="band8")
        nc.gpsimd.dma_start(out=band8_t[:], in_=band8_ap[:, :])

        # Persistent PSUM tiles (same memref -> PE WAW stays program-order).
        psA_tiles = [
            psA_pool.tile([P, BANK], f32, tag="psA", name=f"psA{i}")
            for i in range(3)
        ]
        psB_tiles = [
            psB_pool.tile([P, half_w], f32, tag="psB", name=f"psB{i}")
            for i in range(2)
        ]
        nA = [0]
        nB = [0]

        def next_psA():
            t = psA_tiles[nA[0] % len(psA_tiles)]
            nA[0] += 1
            return t

        def next_psB():
            t = psB_tiles[nB[0] % len(psB_tiles)]
            nB[0] += 1
            return t

        # Input: contiguous row strips, masked to fp8 {0,1}.  Masks split
        # between DVE and the otherwise-idle GpSimd (1-input ~= line rate).
        m_tiles = []
        for kt in range(nt_h):
            xf = xf_pool.tile([P, w], f32)
            nc.sync.dma_start(out=xf[:], in_=x_ap[kt * P : (kt + 1) * P, :])
            m = m_pool.tile([P, w], fp8)
            nc.vector.tensor_scalar(
                m[:], xf[:], 0.0, None, mybir.AluOpType.is_equal
            )
            m_tiles.append(m)

        # s1 strips: S1^T[wt] [w', h] in fp8, written band-by-band.
        s1_tiles = [
            s1_pool.tile([P, h], fp8, tag="s1", name=f"s1_{wt}")
            for wt in range(nt_w)
        ]
        yt_tiles = {}
        done_ht = {}

        # PSUM evacuations alternate DVE / ACT for engine balance.
        ew_flip = [0]

        def evac(dst_ap, src_ap):
            ew_flip[0] += 1
            if ew_flip[0] % 2 == 0:
                nc.vector.tensor_scalar(
                    dst_ap, src_ap, 0.5, None, mybir.AluOpType.is_gt
                )
            else:
                nc.scalar.sign(dst_ap, src_ap)

        thresh = evac

        for bi, (blo, bhi) in enumerate(bands):
            bw = bhi - blo
            # Pass 1 for this band: psum cols are h_out in [blo, bhi)
            for wt in range(nt_w):
                psA = next_psA()
                for kt, lo, hi, st, sp in ph_by_band[bi]:
                    base = P * kt - RADIUS
                    nc.tensor.matmul(
                        psA[:, lo - blo : hi - blo],
                        m_tiles[kt][:, wt * P : (wt + 1) * P],
                        band8_t[:, lo - base : hi - base],
                        start=st,
                        stop=sp,
                    )
                evac(s1_tiles[wt][:, blo:bhi], psA[:, :bw])

            # Pass 2 for the band's output row-strips
            for ht in range(blo // P, bhi // P):
                og, a = divmod(ht, ogrp)
                if og not in yt_tiles:
                    yt_tiles[og] = out_pool.tile(
                        [P, ogrp * w], u8, tag="yt", name=f"yt{og}"
                    )
                    done_ht[og] = 0
                yt = yt_tiles[og]
                for hf in range(n_half):
                    psB = next_psB()
                    for wt, lo, hi, st, sp in pw_by_half[hf]:
                        base = P * wt - RADIUS
                        nc.tensor.matmul(
                            psB[:, lo - hf * half_w : hi - hf * half_w],
                            s1_tiles[wt][:, ht * P : (ht + 1) * P],
                            band8_t[:, lo - base : hi - base],
                            start=st,
                            stop=sp,
                        )
                    thresh(
                        yt[:, a * w + hf * half_w : a * w + (hf + 1) * half_w],
                        psB[:, :half_w],
                    )
                done_ht[og] += 1
                if done_ht[og] == ogrp:
                    dst = y_ap[og * ogrp * P : (og + 1) * ogrp * P, :].rearrange(
                        "(a p) w -> p a w", p=P
                    )
                    nc.gpsimd.dma_start(
                        out=dst, in_=yt[:].rearrange("p (a w) -> p a w", a=ogrp)
                    )

    _dedupe_ldweights(nc)
    nc.compile()
    return nc


def kernel(x: np.ndarray) -> np.ndarray:
    """Full-input entry point: x (8, 2048, 2048, 1) f32 -> y same shape uint8."""
    from concourse.bass_utils import run_bass_kernel_spmd

    x = np.asarray(x)
    assert x.shape == (N_CORES, H, W, 1), x.shape
    imgs = np.ascontiguousarray(x[:, :, :, 0], dtype=np.float32)

    nc = build_program(H, W)
    band8 = band_np().astype(ml_dtypes.float8_e4m3)
    in_maps = [{"x": imgs[c], "band8": band8} for c in range(N_CORES)]
    res = run_bass_kernel_spmd(nc, in_maps, list(range(N_CORES)))
    y = np.stack([res.results[c]["y"] for c in range(N_CORES)])
    return y[..., None]


# revision 41
# speedup vs baseline: 3.1769x; 1.0235x over previous
"""Trainium2 Bass kernel for nn_DilatedMask: 33x33 binary mask dilation.

Computes, for x of shape (8, 2048, 2048, 1) float32:
    mask = (x == 0)
    y    = sliding-window max of mask over a 33x33 window (SAME padding),
           as uint8.

Strategy (per NeuronCore, pure data parallel over the batch of 8):
  A square max window over a binary mask equals (2D box-sum of mask) > 0,
  and the box sum is separable.  Both 1-D 33-wide box sums run on the
  TensorEngine as banded matmuls with the image tile as the *stationary*
  operand, which transposes each pass's output: pass 1 (H-axis sum) emits
  a transposed intermediate [w, h]; pass 2 (W-axis sum over that) lands
  back in natural [h, w] orientation -- no explicit transposes.

    mask  = (x == 0)                              (DVE, f32 -> fp8 {0,1})
    S1^T  = banded sum over H of mask, out [w,h]  (PE, fp8 matmuls)
    S1b   = sign(S1^T) in {0,1}                   (DVE/ACT, PSUM -> fp8)
    S2    = banded sum over W of S1b, out [h,w]   (PE, fp8 matmuls)
    y     = (S2 > 0.5) as uint8                   (DVE/ACT, PSUM -> SBUF)

The H dimension is processed in progressive output bands so pass-2 work
for early rows interleaves with pass-1 of later rows and with the input
DMA stream; the TensorEngine goes dense almost immediately instead of
waiting for the whole image.

PSUM accumulation: within each 512-col PSUM bank the first matmul piece
uses start=True (bank reset), the last stop=True, everything between
accumulates via the per-element has_written bits.
"""

from contextlib import ExitStack

import numpy as np
import ml_dtypes

RADIUS = 16
SE = 2 * RADIUS + 1  # 33
P = 128
BANDW = P + 2 * RADIUS  # 160: out-columns reachable from one 128-row k-tile
BANK = 512  # PSUM bank width in f32 elements
H = W = 2048
N_CORES = 8

# Progressive H output bands: early bands are narrow so pass-1/pass-2 can
# start after only a couple of input row-strips have arrived.
BANDS_H = [(0, 128), (128, 256), (256, 512), (512, 768), (768, 1024),
           (1024, 1536), (1536, 2048)]


def band_np() -> np.ndarray:
    """Band matrix chunk [128, 160]: band[p, j] = 1 iff j-32 <= p <= j."""
    p = np.arange(P)[:, None]
    j = np.arange(BANDW)[None, :]
    return ((p <= j) & (p >= j - 2 * RADIUS)).astype(np.float32)


def _split_at(lo: int, hi: int, cuts):
    out = []
    for c in cuts:
        if lo < c < hi:
            out.append((lo, c))
            lo = c
    out.append((lo, hi))
    return out


def _pieces_for_pass(n: int, extra_cuts=()):
    """Matmul pieces for one banded-sum pass with n output columns.

    Pieces are split at PSUM bank boundaries (512) and any extra cuts.
    Returns list of (kt, lo, hi, start, stop): per physical 512-bank the
    first piece carries start=True, the last stop=True.
    """
    cuts = sorted(set(range(BANK, n, BANK)) | set(extra_cuts))
    nt = n // P
    raw = []  # (kt, lo, hi)
    for kt in range(nt):
        # One merged window per k-tile: [128kt-16, 128kt+144).  Where it
        # overlaps the previous k-tile's window the PSUM has_written bits
        # make the matmul accumulate per element; fresh columns overwrite.
        win_lo = max(0, P * kt - RADIUS)
        win_hi = min(n, P * kt + P + RADIUS)
        for lo, hi in _split_at(win_lo, win_hi, cuts):
            raw.append((kt, lo, hi))
    return raw


def _flag_pieces(raw, tile_base):
    """Assign PSUM start/stop flags for one destination tile's piece list.

    Bank key is relative to the tile base (the tile is bank-aligned); per
    bank the first piece gets start=True, the last stop=True.
    """
    first_in_bank = {}
    last_in_bank = {}
    for i, (kt, lo, hi) in enumerate(raw):
        b = (lo - tile_base) // BANK
        if b not in first_in_bank:
            first_in_bank[b] = i
        last_in_bank[b] = i
    return [
        (kt, lo, hi,
         i == first_in_bank[(lo - tile_base) // BANK],
         i == last_in_bank[(lo - tile_base) // BANK])
        for i, (kt, lo, hi) in enumerate(raw)
    ]


def _dedupe_ldweights(nc):
    """Remove back-to-back duplicate LDWEIGHTS in the PE stream.

    Tile lowers every matmul to LDWEIGHTS+MATMUL; consecutive matmuls that
    share a stationary (the fresh/accumulate piece pairs here) reload the
    identical weights.  The PE pairs each MATMUL with the most recent
    preceding LDWEIGHTS, so the reload is dead -- and LDWEIGHTS streaming
    is this kernel's PE bottleneck.  Only drops loads with empty sync_info.
    """
    import concourse.mybir as mybir

    for blk in nc.m.functions[0].blocks:
        insts = list(blk.instructions)
        keep = []
        remap = {}
        last_sig = None
        for i in insts:
            if i.engine == mybir.EngineType.PE:
                tn = type(i).__name__
                if tn == "InstLdweights":
                    ap = i.ins[0]
                    sig = (ap.memref, ap.offset, str(ap.ap), str(ap.dtype),
                           bool(i.is_transpose), str(i.perf_mode))
                    si = i.sync_info
                    clean = si is None or (
                        not si.on_wait and not si.on_update
                    )
                    if sig == last_sig and clean:
                        remap[i.name] = last_name
                        continue
                    last_sig = sig
                    last_name = i.name
                elif tn != "InstMatmult":
                    last_sig = None
            keep.append(i)
        if remap:
            for i in keep:
                i.remap_dependency_names(remap)
            blk.instructions = keep


def build_program(h: int = H, w: int = W):
    """Build the per-core Bass program (SPMD, identical on all cores)."""
    import concourse.bass as bass
    import concourse.mybir as mybir
    import concourse.tile as tile
    from concourse import bacc

    f32 = mybir.dt.float32
    fp8 = mybir.dt.float8e4
    u8 = mybir.dt.uint8

    nt_h = h // P
    nt_w = w // P
    bands = BANDS_H if h == H else [(0, h)] if h <= 512 else BANDS_H
    if h != H:
        bands = [(a, min(b, h)) for a, b in BANDS_H if a < h]
    n_half = 2 if w > 1024 else 1
    half_w = w // n_half

    nc = bacc.Bacc("TRN2", target_bir_lowering=False, debug=False)
    x_ap = nc.dram_tensor("x", [h, w], f32, kind="ExternalInput").ap()
    band8_ap = nc.dram_tensor("band8", [P, BANDW], fp8, kind="ExternalInput").ap()
    y_ap = nc.dram_tensor("y", [h, w], u8, kind="ExternalOutput").ap()

    band_edges = sorted({a for a, _ in bands} | {b for _, b in bands})
    pieces_h = _pieces_for_pass(h, extra_cuts=band_edges)
    pieces_w = _pieces_for_pass(w, extra_cuts=[half_w] if n_half > 1 else ())
    ph_by_band = {
        bi: _flag_pieces(
            [p for p in pieces_h if blo <= p[1] < bhi], tile_base=blo
        )
        for bi, (blo, bhi) in enumerate(bands)
    }
    pw_by_half = {
        hf: _flag_pieces(
            [p for p in pieces_w if hf * half_w <= p[1] < (hf + 1) * half_w],
            tile_base=hf * half_w,
        )
        for hf in range(n_half)
    }

    OGRP = 2  # output row-strips per store DMA
    n_ogrp = max(1, nt_h // OGRP)
    ogrp = nt_h // n_ogrp

    with tile.TileContext(nc) as tc, ExitStack() as ctx:
        band_pool = ctx.enter_context(tc.tile_pool(name="band", bufs=1))
        xf_pool = ctx.enter_context(tc.tile_pool(name="xf", bufs=6))
        m_pool = ctx.enter_context(tc.tile_pool(name="m", bufs=nt_h))
        s1_pool = ctx.enter_context(tc.tile_pool(name="s1", bufs=nt_w))
        psA_pool = ctx.enter_context(tc.tile_pool(name="psA", bufs=3, space="PSUM"))
        psB_pool = ctx.enter_context(tc.tile_pool(name="psB", bufs=2, space="PSUM"))
        out_pool = ctx.enter_context(tc.tile_pool(name="out", bufs=4))

        band8_t = band_pool.tile([P, BANDW], fp8, tag="band8")
        nc.gpsimd.dma_start(out=band8_t[:], in_=band8_ap[:, :])

        # Persistent PSUM tiles (same memref -> PE WAW stays program-order).
        psA_tiles = [
            psA_pool.tile([P, BANK], f32, tag="psA", name=f"psA{i}")
            for i in range(3)
        ]
        psB_tiles = [
            psB_pool.tile([P, half_w], f32, tag="psB", name=f"psB{i}")
            for i in range(2)
        ]
        nA = [0]
        nB = [0]

        def next_psA():
            t = psA_tiles[nA[0] % len(psA_tiles)]
            nA[0] += 1
            return t

        def next_psB():
            t = psB_tiles[nB[0] % len(psB_tiles)]
            nB[0] += 1
            return t

        # Input: contiguous row strips, masked to fp8 {0,1}.  Masks split
        # between DVE and the otherwise-idle GpSimd (1-input ~= line rate).
        m_tiles = []
        for kt in range(nt_h):
            xf = xf_pool.tile([P, w], f32)
            nc.sync.dma_start(out=xf[:], in_=x_ap[kt * P : (kt + 1) * P, :])
            m = m_pool.tile([P, w], fp8)
            nc.vector.tensor_scalar(
                m[:], xf[:], 0.0, None, mybir.AluOpType.is_equal
            )
            m_tiles.append(m)

        # s1 strips: S1^T[wt] [w', h] in fp8, written band-by-band.
        s1_tiles = [
            s1_pool.tile([P, h], fp8, tag="s1", name=f"s1_{wt}")
            for wt in range(nt_w)
        ]
        yt_tiles = {}
        done_ht = {}

        # PSUM evacuations alternate DVE / ACT for engine balance.
        ew_flip = [0]

        def evac(dst_ap, src_ap):
            ew_flip[0] += 1
            if ew_flip[0] % 2 == 0:
                nc.vector.tensor_scalar(
                    dst_ap, src_ap, 0.5, None, mybir.AluOpType.is_gt
                )
            else:
                nc.scalar.sign(dst_ap, src_ap)

        thresh = evac

        for bi, (blo, bhi) in enumerate(bands):
            bw = bhi - blo
            # Pass 1 for this band: psum cols are h_out in [blo, bhi)
            for wt in range(nt_w):
                psA = next_psA()
                for kt, lo, hi, st, sp in ph_by_band[bi]:
                    base = P * kt - RADIUS
                    nc.tensor.matmul(
                        psA[:, lo - blo : hi - blo],
                        m_tiles[kt][:, wt * P : (wt + 1) * P],
                        band8_t[:, lo - base : hi - base],
                        start=st,
                        stop=sp,
                    )
                evac(s1_tiles[wt][:, blo:bhi], psA[:, :bw])

            # Pass 2 for the band's output row-strips
            for ht in range(blo // P, bhi // P):
                og, a = divmod(ht, ogrp)
                if og not in yt_tiles:
                    yt_tiles[og] = out_pool.tile(
                        [P, ogrp * w], u8, tag="yt", name=f"yt{og}"
                    )
                    done_ht[og] = 0
                yt = yt_tiles[og]
                for hf in range(n_half):
                    psB = next_psB()
                    for wt, lo, hi, st, sp in pw_by_half[hf]:
                        base = P * wt - RADIUS
                        nc.tensor.matmul(
                            psB[:, lo - hf * half_w : hi - hf * half_w],
                            s1_tiles[wt][:, ht * P : (ht + 1) * P],
                            band8_t[:, lo - base : hi - base],
                            start=st,
                            stop=sp,
                        )
                    thresh(
                        yt[:, a * w + hf * half_w : a * w + (hf + 1) * half_w],
                        psB[:, :half_w],
                    )
                done_ht[og] += 1
                if done_ht[og] == ogrp:
                    dst = y_ap[og * ogrp * P : (og + 1) * ogrp * P, :].rearrange(
                        "(a p) w -> p a w", p=P
                    )
                    nc.gpsimd.dma_start(
                        out=dst, in_=yt[:].rearrange("p (a w) -> p a w", a=ogrp)
                    )

    _dedupe_ldweights(nc)
    nc.compile()
    return nc


def kernel(x: np.ndarray) -> np.ndarray:
    """Full-input entry point: x (8, 2048, 2048, 1) f32 -> y same shape uint8."""
    from concourse.bass_utils import run_bass_kernel_spmd

    x = np.asarray(x)
    assert x.shape == (N_CORES, H, W, 1), x.shape
    imgs = np.ascontiguousarray(x[:, :, :, 0], dtype=np.float32)

    nc = build_program(H, W)
    band8 = band_np().astype(ml_dtypes.float8_e4m3)
    in_maps = [{"x": imgs[c], "band8": band8} for c in range(N_CORES)]
    res = run_bass_kernel_spmd(nc, in_maps, list(range(N_CORES)))
    y = np.stack([res.results[c]["y"] for c in range(N_CORES)])
    return y[..., None]
